# revision 4
# baseline (speedup 1.0000x reference)
"""NT-Xent (SimCLR) contrastive loss on 8 Trainium2 NeuronCores.

Strategy (fully SPMD, no collectives):
  z = normalize(concat(emb_i, emb_j))  # [8192, 512]
  Each core c handles a 1024-row block of z. Inputs are pre-rotated on the
  host (np.roll by -c*1024 rows) so every core runs the identical program on
  rows 0..1023 of its own rotated copy: positive pair of rotated row i is
  rotated row (i + 4096) % 8192 for every core.

  Per core:
    - normalize all 8192 rows (fp32 norms via fused DVE square+reduce;
      1/sqrt computed as exp(-0.5*ln) so ACT stays on one table set)
    - stage normalized bf16 z to DRAM, then 32 large DMA-xbar transposes
      (one per d-tile x 1024-row octant) build zT [512, 8192] in SBUF
    - sim row-block = zT[:, :1024].T @ zT in [128, 4x512] 4-bank PSUM tiles
      (bf16 matmul); one ACT exp(2*sim) over 2048 elems with free-dim
      accumulation per tile -> row denominators (exp matrix never stored)
    - self-dot and positive-pair dot per row via fused DVE multiply+reduce
    - loss_row = ln(denom - exp(2*selfdot)) - 2*posdot
  Host: gather 8x1024 row losses, mean.
"""

import numpy as np

import concourse.bacc as bacc
import concourse.tile as tile
from concourse import mybir
from concourse.bass_utils import run_bass_kernel_spmd

N_CORES = 8
D = 512
ROWS = 8192
BLK = ROWS // N_CORES  # 1024
P = 128
N_CHUNKS = ROWS // P  # 64 row-chunks of 128
BLK_CHUNKS = BLK // P  # 8
GROUP = 8  # chunks per octant/norm batch
NT = 512  # one PSUM bank of fp32
N_NT = ROWS // NT  # 16
NG = 4  # n-chunks fused per PSUM tile (4 banks)
KD = D // P  # 4 contraction tiles

f32 = mybir.dt.float32
bf16 = mybir.dt.bfloat16

_ACT_PATCHED = False


def _patch_act_tables():
    """Make Exp and Ln resolve only to natural_log_exp_and_others so the
    whole kernel uses a single activation-table set (one ~2.7us load instead
    of one per Ln<->Exp alternation). Preserves dict order so the emitted
    act_func_set_id indices stay aligned with act_info.json."""
    global _ACT_PATCHED
    if _ACT_PATCHED:
        return
    import concourse.hw_specs as hw_specs

    Act = mybir.ActivationFunctionType
    orig = hw_specs.get_activation_tables("gen3")
    patched = {}
    for name, funcs in orig.items():
        fs = set(funcs)
        if name != "natural_log_exp_and_others":
            fs.discard(Act.Exp)
            fs.discard(Act.Ln)
        patched[name] = fs
    bacc.get_activation_tables = lambda arch: patched
    _ACT_PATCHED = True


def _build():
    Alu = mybir.AluOpType
    Act = mybir.ActivationFunctionType

    _patch_act_tables()
    nc = bacc.Bacc("TRN2", target_bir_lowering=False)
    emb = nc.dram_tensor("emb", [ROWS, D], f32, kind="ExternalInput")
    loss = nc.dram_tensor("loss", [P, BLK_CHUNKS], f32, kind="ExternalOutput")

    with tile.TileContext(nc) as tc:
        with (
            tc.tile_pool(name="persist", bufs=1) as persist,
            tc.tile_pool(name="loads", bufs=16) as loads,
            tc.tile_pool(name="zbgs", bufs=2) as zbgs,
            tc.tile_pool(name="scratch", bufs=3) as scratch,
            tc.tile_pool(name="small", bufs=2) as small,
            tc.tile_pool(name="dram", bufs=1, space="DRAM") as dram,
            tc.tile_pool(name="psum", bufs=2, space="PSUM") as psum_pool,
        ):
            # persistent tensors
            zT = [
                persist.tile([P, ROWS], bf16, tag=f"zT{k}", name=f"zT{k}")
                for k in range(KD)
            ]
            acc = [
                persist.tile([P, N_NT // NG], f32, tag=f"acc{m}", name=f"acc{m}")
                for m in range(BLK_CHUNKS)
            ]
            selfd = persist.tile([P, BLK_CHUNKS], f32, tag="selfd")
            posd = persist.tile([P, BLK_CHUNKS], f32, tag="posd")
            zbd = dram.tile([ROWS, D], bf16, tag="zbd", name="zbd")  # staged z

            # octant 0 holds the block rows, octant 4 the positive pairs;
            # process those first so the main loop can start early.
            octant_order = [0, 4, 1, 2, 3, 5, 6, 7]
            zbg_keep = {}
            for oct_ in octant_order:
                sq = small.tile([P, GROUP], f32, tag="sq")
                ets = []
                for i in range(GROUP):
                    j = oct_ * GROUP + i
                    et = loads.tile([P, D], f32, tag="et")
                    nc.sync.dma_start(out=et, in_=emb[j * P : (j + 1) * P, :])
                    tt = scratch.tile([P, D], bf16, tag="ttout")
                    nc.vector.scalar_tensor_tensor(
                        out=tt,
                        in0=et,
                        scalar=1.0,
                        in1=et,
                        op0=Alu.mult,
                        op1=Alu.mult,
                        accum_out=sq[:, i : i + 1],
                    )
                    ets.append(et)
                # 1/sqrt(x) = exp(-0.5 * ln(x)) -- single ACT table set
                lnv = small.tile([P, GROUP], f32, tag="lnv")
                nc.scalar.activation(out=lnv, in_=sq, func=Act.Ln)
                rinv = small.tile([P, GROUP], f32, tag="rinv")
                nc.scalar.activation(out=rinv, in_=lnv, func=Act.Exp, scale=-0.5)

                if oct_ in (0, 4):
                    zbg = persist.tile(
                        [P, GROUP, D], bf16, tag=f"zbg{oct_}", name=f"zbg{oct_}"
                    )
                    zbg_keep[oct_] = zbg
                else:
                    zbg = zbgs.tile([P, GROUP, D], bf16, tag="zbg")
                for i in range(GROUP):
                    nc.vector.tensor_scalar_mul(
                        out=zbg[:, i, :], in0=ets[i], scalar1=rinv[:, i : i + 1]
                    )
                # stage octant (1 MiB) to DRAM on the SWDGE path
                dst = zbd[oct_ * BLK : (oct_ + 1) * BLK, :].rearrange(
                    "(c p) d -> p c d", p=P
                )
                nc.gpsimd.dma_start(out=dst, in_=zbg)
                # large xbar transposes DRAM -> zT columns for this octant
                for k in range(KD):
                    nc.sync.dma_start_transpose(
                        out=zT[k][:, oct_ * BLK : (oct_ + 1) * BLK],
                        in_=zbd[oct_ * BLK : (oct_ + 1) * BLK, k * P : (k + 1) * P],
                    )

                if oct_ == 4:
                    # blk + pos rows normalized: per-row self/pos dots
                    for m in range(BLK_CHUNKS):
                        t1 = scratch.tile([P, D], bf16, tag="ttout")
                        nc.vector.scalar_tensor_tensor(
                            out=t1,
                            in0=zbg_keep[0][:, m, :],
                            scalar=1.0,
                            in1=zbg_keep[0][:, m, :],
                            op0=Alu.mult,
                            op1=Alu.mult,
                            accum_out=selfd[:, m : m + 1],
                        )
                        t2 = scratch.tile([P, D], bf16, tag="ttout")
                        nc.vector.scalar_tensor_tensor(
                            out=t2,
                            in0=zbg_keep[0][:, m, :],
                            scalar=1.0,
                            in1=zbg_keep[4][:, m, :],
                            op0=Alu.mult,
                            op1=Alu.mult,
                            accum_out=posd[:, m : m + 1],
                        )

            # main loop: 4 n-chunks share a 4-bank PSUM tile; k-inner so one
            # stationary operand serves 4 consecutive matmuls.
            n_groups = [[0, 1, 8, 9], [2, 3, 4, 5], [6, 7, 10, 11], [12, 13, 14, 15]]
            for ng, group in enumerate(n_groups):
                for m in range(BLK_CHUNKS):
                    pst = psum_pool.tile([P, NG, NT], f32, tag="ps")
                    for k in range(KD):
                        for li, n in enumerate(group):
                            nc.tensor.matmul(
                                pst[:, li, :],
                                zT[k][:, m * P : (m + 1) * P],
                                zT[k][:, n * NT : (n + 1) * NT],
                                start=(k == 0),
                                stop=(k == KD - 1),
                            )
                    ex = scratch.tile([P, NG, NT], bf16, tag="exout")
                    nc.scalar.activation(
                        out=ex,
                        in_=pst,
                        func=Act.Exp,
                        scale=2.0,
                        accum_out=acc[m][:, ng : ng + 1],
                    )

            # finale: loss_row = ln(denom - exp(2*selfdot)) - 2*posdot
            dsum = persist.tile([P, BLK_CHUNKS], f32, tag="dsum")
            for m in range(BLK_CHUNKS):
                nc.vector.reduce_sum(
                    out=dsum[:, m : m + 1], in_=acc[m], axis=mybir.AxisListType.X
                )
            sexp = small.tile([P, BLK_CHUNKS], f32, tag="sexp")
            nc.scalar.activation(out=sexp, in_=selfd, func=Act.Exp, scale=2.0)
            dx = small.tile([P, BLK_CHUNKS], f32, tag="dx")
            nc.vector.tensor_sub(dx, dsum, sexp)
            ld = small.tile([P, BLK_CHUNKS], f32, tag="ld")
            nc.scalar.activation(out=ld, in_=dx, func=Act.Ln)
            lossv = small.tile([P, BLK_CHUNKS], f32, tag="lossv")
            nc.vector.scalar_tensor_tensor(
                out=lossv,
                in0=posd,
                scalar=-2.0,
                in1=ld,
                op0=Alu.mult,
                op1=Alu.add,
            )
            nc.sync.dma_start(out=loss[:, :], in_=lossv)

    nc.compile()
    return nc


_NC_CACHE = []


def _get_nc():
    if not _NC_CACHE:
        _NC_CACHE.append(_build())
    return _NC_CACHE[0]


def make_in_maps(emb_i: np.ndarray, emb_j: np.ndarray):
    emb_all = np.concatenate(
        [np.asarray(emb_i, np.float32), np.asarray(emb_j, np.float32)], axis=0
    )
    return [
        {"emb": np.ascontiguousarray(np.roll(emb_all, -c * BLK, axis=0))}
        for c in range(N_CORES)
    ]


def assemble(results) -> np.ndarray:
    rows = []
    for c in range(N_CORES):
        out = results[c]["loss"]  # [128, 8]; out[p, m] = loss of block row m*128+p
        rows.append(out.T.reshape(-1))
    all_rows = np.concatenate(rows)  # original row order
    return np.float32(all_rows.astype(np.float64).mean())


def kernel(emb_i: np.ndarray, emb_j: np.ndarray) -> np.ndarray:
    nc = _get_nc()
    res = run_bass_kernel_spmd(
        nc, make_in_maps(emb_i, emb_j), core_ids=list(range(N_CORES))
    )
    return assemble(res.results)


if __name__ == "__main__":
    rng = np.random.default_rng(0)
    ei = rng.standard_normal((4096, D)).astype(np.float32)
    ej = rng.standard_normal((4096, D)).astype(np.float32)
    print(kernel(ei, ej))


# revision 9
# speedup vs baseline: 1.1905x; 1.1905x over previous
"""NT-Xent (SimCLR) contrastive loss on 8 Trainium2 NeuronCores.

Strategy (fully SPMD, no collectives):
  z = normalize(concat(emb_i, emb_j))  # [8192, 512]
  Each core c handles a 1024-row block of z. Inputs are pre-rotated on the
  host (np.roll by -c*1024 rows) so every core runs the identical program on
  rows 0..1023 of its own rotated copy: positive pair of rotated row i is
  rotated row (i + 4096) % 8192 for every core.

  Per core:
    - normalize all 8192 rows (fp32 norms via fused DVE square+reduce;
      1/sqrt computed as exp(-0.5*ln) so ACT stays on one table set)
    - stage normalized bf16 z to DRAM, then 32 large DMA-xbar transposes
      (one per d-tile x 1024-row octant) build zT [512, 8192] in SBUF
    - sim row-block = zT[:, :1024].T @ zT in [128, 4x512] 4-bank PSUM tiles
      (bf16 matmul); one ACT exp(2*sim) over 2048 elems with free-dim
      accumulation per tile -> row denominators (exp matrix never stored)
    - self-dot and positive-pair dot per row via fused DVE multiply+reduce
    - loss_row = ln(denom - exp(2*selfdot)) - 2*posdot
  Host: gather 8x1024 row losses, mean.
"""

import numpy as np

import concourse.bacc as bacc
import concourse.tile as tile
from concourse import mybir
from concourse.bass_utils import run_bass_kernel_spmd

N_CORES = 8
D = 512
ROWS = 8192
BLK = ROWS // N_CORES  # 1024
P = 128
N_CHUNKS = ROWS // P  # 64 row-chunks of 128
BLK_CHUNKS = BLK // P  # 8
GROUP = 8  # chunks per octant/norm batch
NT = 512  # one PSUM bank of fp32
N_NT = ROWS // NT  # 16
NG = 4  # n-chunks fused per PSUM tile (4 banks)
KD = D // P  # 4 contraction tiles

f32 = mybir.dt.float32
bf16 = mybir.dt.bfloat16

_ACT_PATCHED = False


def _patch_act_tables():
    """Make Exp and Ln resolve only to natural_log_exp_and_others so the
    whole kernel uses a single activation-table set (one ~2.7us load instead
    of one per Ln<->Exp alternation). Preserves dict order so the emitted
    act_func_set_id indices stay aligned with act_info.json."""
    global _ACT_PATCHED
    if _ACT_PATCHED:
        return
    import concourse.hw_specs as hw_specs

    Act = mybir.ActivationFunctionType
    orig = hw_specs.get_activation_tables("gen3")
    patched = {}
    for name, funcs in orig.items():
        fs = set(funcs)
        if name != "natural_log_exp_and_others":
            fs.discard(Act.Exp)
            fs.discard(Act.Ln)
        patched[name] = fs
    bacc.get_activation_tables = lambda arch: patched
    _ACT_PATCHED = True


def _build():
    Alu = mybir.AluOpType
    Act = mybir.ActivationFunctionType

    _patch_act_tables()
    nc = bacc.Bacc("TRN2", target_bir_lowering=False)
    emb = nc.dram_tensor("emb", [ROWS, D], f32, kind="ExternalInput")
    loss = nc.dram_tensor("loss", [P, BLK_CHUNKS], f32, kind="ExternalOutput")

    with tile.TileContext(nc) as tc:
        with (
            tc.tile_pool(name="persist", bufs=1) as persist,
            tc.tile_pool(name="loads", bufs=4) as loads,
            tc.tile_pool(name="zbgs", bufs=2) as zbgs,
            tc.tile_pool(name="scratch", bufs=3) as scratch,
            tc.tile_pool(name="small", bufs=2) as small,
            tc.tile_pool(name="dram", bufs=1, space="DRAM") as dram,
            tc.tile_pool(name="psum", bufs=2, space="PSUM") as psum_pool,
        ):
            # persistent tensors
            zT = [
                persist.tile([P, ROWS], bf16, tag=f"zT{k}", name=f"zT{k}")
                for k in range(KD)
            ]
            acc = [
                persist.tile([P, 4], f32, tag=f"acc{m}", name=f"acc{m}")
                for m in range(BLK_CHUNKS)
            ]
            selfd = persist.tile([P, BLK_CHUNKS], f32, tag="selfd")
            posd = persist.tile([P, BLK_CHUNKS], f32, tag="posd")
            zbd = dram.tile([ROWS, D], bf16, tag="zbd", name="zbd")  # staged z

            # octant 0 holds the block rows, octant 4 the positive pairs;
            # process those first so the main loop can start early.
            octant_order = [0, 4, 1, 2, 3, 5, 6, 7]
            zbg_keep = {}
            for oct_ in octant_order:
                sq = small.tile([P, GROUP], f32, tag="sq")
                et4s = []
                for h in range(GROUP // 4):
                    et4 = loads.tile([P, 4, D], f32, tag="et4")
                    r0 = (oct_ * GROUP + 4 * h) * P
                    src = emb[r0 : r0 + 4 * P, :].rearrange("(c p) d -> p c d", p=P)
                    nc.sync.dma_start(out=et4, in_=src)
                    et4s.append(et4)
                ets = [et4s[i // 4][:, i % 4, :] for i in range(GROUP)]
                for i in range(GROUP):
                    tt = scratch.tile([P, D], bf16, tag="ttout")
                    nc.vector.scalar_tensor_tensor(
                        out=tt,
                        in0=ets[i],
                        scalar=1.0,
                        in1=ets[i],
                        op0=Alu.mult,
                        op1=Alu.mult,
                        accum_out=sq[:, i : i + 1],
                    )
                # 1/sqrt(x) = exp(-0.5 * ln(x)) -- single ACT table set
                lnv = small.tile([P, GROUP], f32, tag="lnv")
                nc.scalar.activation(out=lnv, in_=sq, func=Act.Ln)
                rinv = small.tile([P, GROUP], f32, tag="rinv")
                nc.scalar.activation(out=rinv, in_=lnv, func=Act.Exp, scale=-0.5)

                if oct_ in (0, 4):
                    zbg = persist.tile(
                        [P, GROUP, D], bf16, tag=f"zbg{oct_}", name=f"zbg{oct_}"
                    )
                    zbg_keep[oct_] = zbg
                else:
                    zbg = zbgs.tile([P, GROUP, D], bf16, tag="zbg")
                for i in range(GROUP):
                    nc.vector.tensor_scalar_mul(
                        out=zbg[:, i, :], in0=ets[i], scalar1=rinv[:, i : i + 1]
                    )
                # stage octant (1 MiB) to DRAM on the SWDGE path
                dst = zbd[oct_ * BLK : (oct_ + 1) * BLK, :].rearrange(
                    "(c p) d -> p c d", p=P
                )
                nc.gpsimd.dma_start(out=dst, in_=zbg)
                # large xbar transposes DRAM -> zT columns for this octant
                for k in range(KD):
                    nc.sync.dma_start_transpose(
                        out=zT[k][:, oct_ * BLK : (oct_ + 1) * BLK],
                        in_=zbd[oct_ * BLK : (oct_ + 1) * BLK, k * P : (k + 1) * P],
                    )

                if oct_ == 4:
                    # blk + pos rows normalized: per-row self/pos dots
                    for m in range(BLK_CHUNKS):
                        t1 = scratch.tile([P, D], bf16, tag="ttout")
                        nc.vector.scalar_tensor_tensor(
                            out=t1,
                            in0=zbg_keep[0][:, m, :],
                            scalar=1.0,
                            in1=zbg_keep[0][:, m, :],
                            op0=Alu.mult,
                            op1=Alu.mult,
                            accum_out=selfd[:, m : m + 1],
                        )
                        t2 = scratch.tile([P, D], bf16, tag="ttout")
                        nc.vector.scalar_tensor_tensor(
                            out=t2,
                            in0=zbg_keep[0][:, m, :],
                            scalar=1.0,
                            in1=zbg_keep[4][:, m, :],
                            op0=Alu.mult,
                            op1=Alu.mult,
                            accum_out=posd[:, m : m + 1],
                        )

            # main loop: 4 n-chunks share a 4-bank PSUM tile; k-inner so one
            # stationary operand serves 4 consecutive matmuls.
            n_groups = [[0, 1, 8, 9], [2, 3, 4, 5], [6, 7, 10, 11], [12, 13, 14, 15]]
            for ng, group in enumerate(n_groups):
                gw = len(group)
                for m in range(BLK_CHUNKS):
                    pst = psum_pool.tile([P, gw, NT], f32, tag="ps", bufs=2)
                    for k in range(KD):
                        for li, n in enumerate(group):
                            nc.tensor.matmul(
                                pst[:, li, :],
                                zT[k][:, m * P : (m + 1) * P],
                                zT[k][:, n * NT : (n + 1) * NT],
                                start=(k == 0),
                                stop=(k == KD - 1),
                            )
                    ex = scratch.tile([P, gw, NT], bf16, tag="exout")
                    nc.scalar.activation(
                        out=ex,
                        in_=pst,
                        func=Act.Exp,
                        scale=2.0,
                        accum_out=acc[m][:, ng : ng + 1],
                    )

            # finale: loss_row = ln(denom - exp(2*selfdot)) - 2*posdot
            dsum = persist.tile([P, BLK_CHUNKS], f32, tag="dsum")
            for m in range(BLK_CHUNKS):
                nc.vector.reduce_sum(
                    out=dsum[:, m : m + 1], in_=acc[m], axis=mybir.AxisListType.X
                )
            sexp = small.tile([P, BLK_CHUNKS], f32, tag="sexp")
            nc.scalar.activation(out=sexp, in_=selfd, func=Act.Exp, scale=2.0)
            dx = small.tile([P, BLK_CHUNKS], f32, tag="dx")
            nc.vector.tensor_sub(dx, dsum, sexp)
            ld = small.tile([P, BLK_CHUNKS], f32, tag="ld")
            nc.scalar.activation(out=ld, in_=dx, func=Act.Ln)
            lossv = small.tile([P, BLK_CHUNKS], f32, tag="lossv")
            nc.vector.scalar_tensor_tensor(
                out=lossv,
                in0=posd,
                scalar=-2.0,
                in1=ld,
                op0=Alu.mult,
                op1=Alu.add,
            )
            nc.sync.dma_start(out=loss[:, :], in_=lossv)

    nc.compile()
    return nc


_NC_CACHE = []


def _get_nc():
    if not _NC_CACHE:
        _NC_CACHE.append(_build())
    return _NC_CACHE[0]


def make_in_maps(emb_i: np.ndarray, emb_j: np.ndarray):
    emb_all = np.concatenate(
        [np.asarray(emb_i, np.float32), np.asarray(emb_j, np.float32)], axis=0
    )
    return [
        {"emb": np.ascontiguousarray(np.roll(emb_all, -c * BLK, axis=0))}
        for c in range(N_CORES)
    ]


def assemble(results) -> np.ndarray:
    rows = []
    for c in range(N_CORES):
        out = results[c]["loss"]  # [128, 8]; out[p, m] = loss of block row m*128+p
        rows.append(out.T.reshape(-1))
    all_rows = np.concatenate(rows)  # original row order
    return np.float32(all_rows.astype(np.float64).mean())


def kernel(emb_i: np.ndarray, emb_j: np.ndarray) -> np.ndarray:
    nc = _get_nc()
    res = run_bass_kernel_spmd(
        nc, make_in_maps(emb_i, emb_j), core_ids=list(range(N_CORES))
    )
    return assemble(res.results)


if __name__ == "__main__":
    rng = np.random.default_rng(0)
    ei = rng.standard_normal((4096, D)).astype(np.float32)
    ej = rng.standard_normal((4096, D)).astype(np.float32)
    print(kernel(ei, ej))


# revision 10
# speedup vs baseline: 3.6776x; 3.0891x over previous
"""NT-Xent (SimCLR) contrastive loss on 8 Trainium2 NeuronCores.

Strategy (fully SPMD, no collectives):
  z = normalize(concat(emb_i, emb_j))  # [8192, 512]
  Each core c handles a 1024-row block of z. Inputs are pre-rotated on the
  host (np.roll by -c*1024 rows) so every core runs the identical program on
  rows 0..1023 of its own rotated copy: positive pair of rotated row i is
  rotated row (i + 4096) % 8192 for every core.

  Per core:
    - normalize all 8192 rows (fp32 norms via fused DVE square+reduce;
      1/sqrt computed as exp(-0.5*ln) so ACT stays on one table set)
    - stage normalized bf16 z to DRAM, then 32 large DMA-xbar transposes
      (one per d-tile x 1024-row octant) build zT [512, 8192] in SBUF
    - sim row-block = zT[:, :1024].T @ zT in [128, 4x512] 4-bank PSUM tiles
      (bf16 matmul); one ACT exp(2*sim) over 2048 elems with free-dim
      accumulation per tile -> row denominators (exp matrix never stored)
    - self-dot and positive-pair dot per row via fused DVE multiply+reduce
    - loss_row = ln(denom - exp(2*selfdot)) - 2*posdot
  Host: gather 8x1024 row losses, mean.
"""

import numpy as np

import concourse.bacc as bacc
import concourse.tile as tile
from concourse import mybir
from concourse.bass_utils import run_bass_kernel_spmd

N_CORES = 8
D = 512
ROWS = 8192
BLK = ROWS // N_CORES  # 1024
P = 128
N_CHUNKS = ROWS // P  # 64 row-chunks of 128
BLK_CHUNKS = BLK // P  # 8
GROUP = 8  # chunks per octant/norm batch
NT = 512  # one PSUM bank of fp32
N_NT = ROWS // NT  # 16
NG = 4  # n-chunks fused per PSUM tile (4 banks)
KD = D // P  # 4 contraction tiles

f32 = mybir.dt.float32
bf16 = mybir.dt.bfloat16

_ACT_PATCHED = False


def _patch_act_tables():
    """Make Exp and Ln resolve only to natural_log_exp_and_others so the
    whole kernel uses a single activation-table set (one ~2.7us load instead
    of one per Ln<->Exp alternation). Preserves dict order so the emitted
    act_func_set_id indices stay aligned with act_info.json."""
    global _ACT_PATCHED
    if _ACT_PATCHED:
        return
    import concourse.hw_specs as hw_specs

    Act = mybir.ActivationFunctionType
    orig = hw_specs.get_activation_tables("gen3")
    patched = {}
    for name, funcs in orig.items():
        fs = set(funcs)
        if name != "natural_log_exp_and_others":
            fs.discard(Act.Exp)
            fs.discard(Act.Ln)
        patched[name] = fs
    bacc.get_activation_tables = lambda arch: patched
    _ACT_PATCHED = True


def _build(loop_k: int = 1):
    Alu = mybir.AluOpType
    Act = mybir.ActivationFunctionType

    _patch_act_tables()
    nc = bacc.Bacc("TRN2", target_bir_lowering=False)
    emb = nc.dram_tensor("emb", [ROWS, D], f32, kind="ExternalInput")
    loss = nc.dram_tensor("loss", [P, BLK_CHUNKS], f32, kind="ExternalOutput")

    with tile.TileContext(nc) as tc:
        with (
            tc.tile_pool(name="persist", bufs=1) as persist,
            tc.tile_pool(name="loads", bufs=4) as loads,
            tc.tile_pool(name="zbgs", bufs=2) as zbgs,
            tc.tile_pool(name="scratch", bufs=3) as scratch,
            tc.tile_pool(name="small", bufs=2) as small,
            tc.tile_pool(name="dram", bufs=1, space="DRAM") as dram,
            tc.tile_pool(name="psum", bufs=2, space="PSUM") as psum_pool,
        ):
            import contextlib

            loop_ctx = (
                tc.For_i(0, loop_k, 1) if loop_k > 1 else contextlib.nullcontext()
            )
            with loop_ctx:
                _body(nc, tc, persist, loads, zbgs, scratch, small, dram, psum_pool, emb, loss)

    nc.compile()
    return nc


def _body(nc, tc, persist, loads, zbgs, scratch, small, dram, psum_pool, emb, loss):
    Alu = mybir.AluOpType
    Act = mybir.ActivationFunctionType
    if True:
        if True:
            # persistent tensors
            zT = [
                persist.tile([P, ROWS], bf16, tag=f"zT{k}", name=f"zT{k}")
                for k in range(KD)
            ]
            acc = [
                persist.tile([P, 4], f32, tag=f"acc{m}", name=f"acc{m}")
                for m in range(BLK_CHUNKS)
            ]
            selfd = persist.tile([P, BLK_CHUNKS], f32, tag="selfd")
            posd = persist.tile([P, BLK_CHUNKS], f32, tag="posd")
            zbd = dram.tile([ROWS, D], bf16, tag="zbd", name="zbd")  # staged z

            # octant 0 holds the block rows, octant 4 the positive pairs;
            # process those first so the main loop can start early.
            octant_order = [0, 4, 1, 2, 3, 5, 6, 7]
            zbg_keep = {}
            for oct_ in octant_order:
                sq = small.tile([P, GROUP], f32, tag="sq")
                et4s = []
                for h in range(GROUP // 4):
                    et4 = loads.tile([P, 4, D], f32, tag="et4")
                    r0 = (oct_ * GROUP + 4 * h) * P
                    src = emb[r0 : r0 + 4 * P, :].rearrange("(c p) d -> p c d", p=P)
                    nc.sync.dma_start(out=et4, in_=src)
                    et4s.append(et4)
                ets = [et4s[i // 4][:, i % 4, :] for i in range(GROUP)]
                for i in range(GROUP):
                    tt = scratch.tile([P, D], bf16, tag="ttout")
                    nc.vector.scalar_tensor_tensor(
                        out=tt,
                        in0=ets[i],
                        scalar=1.0,
                        in1=ets[i],
                        op0=Alu.mult,
                        op1=Alu.mult,
                        accum_out=sq[:, i : i + 1],
                    )
                # 1/sqrt(x) = exp(-0.5 * ln(x)) -- single ACT table set
                lnv = small.tile([P, GROUP], f32, tag="lnv")
                nc.scalar.activation(out=lnv, in_=sq, func=Act.Ln)
                rinv = small.tile([P, GROUP], f32, tag="rinv")
                nc.scalar.activation(out=rinv, in_=lnv, func=Act.Exp, scale=-0.5)

                if oct_ in (0, 4):
                    zbg = persist.tile(
                        [P, GROUP, D], bf16, tag=f"zbg{oct_}", name=f"zbg{oct_}"
                    )
                    zbg_keep[oct_] = zbg
                else:
                    zbg = zbgs.tile([P, GROUP, D], bf16, tag="zbg")
                for i in range(GROUP):
                    nc.vector.tensor_scalar_mul(
                        out=zbg[:, i, :], in0=ets[i], scalar1=rinv[:, i : i + 1]
                    )
                # stage octant (1 MiB) to DRAM on the SWDGE path
                dst = zbd[oct_ * BLK : (oct_ + 1) * BLK, :].rearrange(
                    "(c p) d -> p c d", p=P
                )
                nc.gpsimd.dma_start(out=dst, in_=zbg)
                # large xbar transposes DRAM -> zT columns for this octant
                for k in range(KD):
                    nc.sync.dma_start_transpose(
                        out=zT[k][:, oct_ * BLK : (oct_ + 1) * BLK],
                        in_=zbd[oct_ * BLK : (oct_ + 1) * BLK, k * P : (k + 1) * P],
                    )

                if oct_ == 4:
                    # blk + pos rows normalized: per-row self/pos dots
                    for m in range(BLK_CHUNKS):
                        t1 = scratch.tile([P, D], bf16, tag="ttout")
                        nc.vector.scalar_tensor_tensor(
                            out=t1,
                            in0=zbg_keep[0][:, m, :],
                            scalar=1.0,
                            in1=zbg_keep[0][:, m, :],
                            op0=Alu.mult,
                            op1=Alu.mult,
                            accum_out=selfd[:, m : m + 1],
                        )
                        t2 = scratch.tile([P, D], bf16, tag="ttout")
                        nc.vector.scalar_tensor_tensor(
                            out=t2,
                            in0=zbg_keep[0][:, m, :],
                            scalar=1.0,
                            in1=zbg_keep[4][:, m, :],
                            op0=Alu.mult,
                            op1=Alu.mult,
                            accum_out=posd[:, m : m + 1],
                        )

            # main loop: 4 n-chunks share a 4-bank PSUM tile; k-inner so one
            # stationary operand serves 4 consecutive matmuls.
            n_groups = [[0, 1, 8, 9], [2, 3, 4, 5], [6, 7, 10, 11], [12, 13, 14, 15]]
            for ng, group in enumerate(n_groups):
                gw = len(group)
                for m in range(BLK_CHUNKS):
                    pst = psum_pool.tile([P, gw, NT], f32, tag="ps", bufs=2)
                    for k in range(KD):
                        for li, n in enumerate(group):
                            nc.tensor.matmul(
                                pst[:, li, :],
                                zT[k][:, m * P : (m + 1) * P],
                                zT[k][:, n * NT : (n + 1) * NT],
                                start=(k == 0),
                                stop=(k == KD - 1),
                            )
                    ex = scratch.tile([P, gw, NT], bf16, tag="exout")
                    nc.scalar.activation(
                        out=ex,
                        in_=pst,
                        func=Act.Exp,
                        scale=2.0,
                        accum_out=acc[m][:, ng : ng + 1],
                    )

            # finale: loss_row = ln(denom - exp(2*selfdot)) - 2*posdot
            dsum = persist.tile([P, BLK_CHUNKS], f32, tag="dsum")
            for m in range(BLK_CHUNKS):
                nc.vector.reduce_sum(
                    out=dsum[:, m : m + 1], in_=acc[m], axis=mybir.AxisListType.X
                )
            sexp = small.tile([P, BLK_CHUNKS], f32, tag="sexp")
            nc.scalar.activation(out=sexp, in_=selfd, func=Act.Exp, scale=2.0)
            dx = small.tile([P, BLK_CHUNKS], f32, tag="dx")
            nc.vector.tensor_sub(dx, dsum, sexp)
            ld = small.tile([P, BLK_CHUNKS], f32, tag="ld")
            nc.scalar.activation(out=ld, in_=dx, func=Act.Ln)
            lossv = small.tile([P, BLK_CHUNKS], f32, tag="lossv")
            nc.vector.scalar_tensor_tensor(
                out=lossv,
                in0=posd,
                scalar=-2.0,
                in1=ld,
                op0=Alu.mult,
                op1=Alu.add,
            )
            nc.sync.dma_start(out=loss[:, :], in_=lossv)


_NC_CACHE = []


def _get_nc():
    if not _NC_CACHE:
        _NC_CACHE.append(_build())
    return _NC_CACHE[0]


def make_in_maps(emb_i: np.ndarray, emb_j: np.ndarray):
    emb_all = np.concatenate(
        [np.asarray(emb_i, np.float32), np.asarray(emb_j, np.float32)], axis=0
    )
    return [
        {"emb": np.ascontiguousarray(np.roll(emb_all, -c * BLK, axis=0))}
        for c in range(N_CORES)
    ]


def assemble(results) -> np.ndarray:
    rows = []
    for c in range(N_CORES):
        out = results[c]["loss"]  # [128, 8]; out[p, m] = loss of block row m*128+p
        rows.append(out.T.reshape(-1))
    all_rows = np.concatenate(rows)  # original row order
    return np.float32(all_rows.astype(np.float64).mean())


def kernel(emb_i: np.ndarray, emb_j: np.ndarray) -> np.ndarray:
    nc = _get_nc()
    res = run_bass_kernel_spmd(
        nc, make_in_maps(emb_i, emb_j), core_ids=list(range(N_CORES))
    )
    return assemble(res.results)


if __name__ == "__main__":
    rng = np.random.default_rng(0)
    ei = rng.standard_normal((4096, D)).astype(np.float32)
    ej = rng.standard_normal((4096, D)).astype(np.float32)
    print(kernel(ei, ej))
